# revision 10
# baseline (speedup 1.0000x reference)
"""Trainium2 Bass kernel for the B-spline (KAN-style) layer.

Computes out[b,f] = sum_k basis_k(x[b,f]) * control_p[k,f] + bias[f] where
basis is the cubic B-spline basis from the reference (64 functions, knots
uniform on [0,1] with spacing 1/55 plus boundary extension knots).

Algorithm: two-level "telescoped clamp" in s = 55*x coordinates (integer
knots).  A C^2 piecewise cubic telescopes into clamped cubics with no
data-dependent lookup:

    S(s) = S(0) + sum_k [coarse cubic_k(clamp(s - 11k, 0, 11))]
                + sum_m e_m * clamp(s - m, 0, K_m)^3

with one width-11 coarse piece per 11 knots (2 fused DVE ops for a,b + c)
and a single-coefficient truncated-power correction at each interior knot m
(clamped at the coarse piece end, K_m in {1..10}).  60 fused 8-stage custom
DVE instructions total.

Unlike the width-5 predecessor, corrections run directly on the raw s tile:
the clamp limit 2*K_m is passed through the second scalar slot
(v = min(t + |t|, C1) = 2*clamp(s-m, 0, K)), so no per-width prescaled
copies of s are needed.  Only the 5 coarse-piece ops use a single s/11 tile
(one ACT mul).  Chain heads are Src1-free op variants (no memsets); the
spline constant + bias rides the C1 slot of one head.

Sharding: data-parallel over batch (4 slices) x features (2 halves) = 8
cores; no collectives.  The host pre-transposes each shard to
(feature, batch) layout and pre-computes the per-feature coefficient table
gtab = [control_p; bias].T @ W2 in float64 (W2 is a fixed host constant),
so the device does no transposes and no table matmul: DMA s + gtab in,
60 chain ops + 3 merge adds (1 on GPSIMD), DMA out.
"""

import sys

if "/opt/trn_rl_repo" not in sys.path:
    sys.path.insert(0, "/opt/trn_rl_repo")

import numpy as np

import concourse.bass as bass
import concourse.bacc as bacc
import concourse.tile as tile
from concourse import mybir
from concourse.bass_utils import run_bass_kernel_spmd

BATCH, NF, NK, DG = 4096, 256, 64, 3
NJ = 55          # spline intervals covering x in [0,1)
NCORES = 8
BSH, FSH = 1024, 128   # per-core shard: batch x features
F32 = mybir.dt.float32

CW = 11                  # coarse piece width
NCP = NJ // CW           # 5 coarse pieces (CW * NCP == NJ)
NCOR = NJ - NCP          # 50 interior-knot corrections
NW2 = 3 * NCP + NCOR + 1  # a,b,c per coarse piece + corrections + const
NACC = 3                 # independent accumulator chains
UNROLL = 4               # bodies per For_i iteration (amortizes the
                         # all-engine back-edge barrier and lets copy u's
                         # DMA/ACT/GPSIMD head+tail overlap copy u-1's DVE)

# ---------------------------------------------------------------------------
# Host-side spline tables (float64, exact)
# ---------------------------------------------------------------------------

def _knots64():
    dg, nk = DG, NK
    base = np.concatenate([
        np.linspace(-0.002, -0.001, dg),
        np.linspace(0.0, 1.0, nk - 2 * dg - 2),
        np.linspace(1.001, 1.002, dg),
    ])
    dist_lo = base[1] - base[0]
    dist_hi = base[-1] - base[-2]
    left = base[0] - dist_lo * np.arange(dg, 0, -1)
    right = base[-1] + dist_hi * np.arange(1, dg + 1)
    t32 = np.concatenate([left, base, right]).astype(np.float32)
    return t32.astype(np.float64)


def _basis64(x, t):
    xe = x[..., None]
    B = ((t[:-1] <= xe) & (xe < t[1:])).astype(np.float64)
    for k in range(1, DG + 1):
        d1 = t[k:-1] - t[:-k - 1]
        d2 = t[k + 1:] - t[1:-k]
        w1 = np.where(d1 != 0, (xe - t[:-k - 1]) / np.where(d1 != 0, d1, 1.0), 0.0)
        w2 = np.where(d2 != 0, (t[k + 1:] - xe) / np.where(d2 != 0, d2, 1.0), 0.0)
        B = w1 * B[..., :-1] + w2 * B[..., 1:]
    return B  # (..., 64)


def _gtable():
    """Per-interval cubic coefficients g[j, d] as linear maps over the 64
    control points: returns (55, 4, 64) float64."""
    t = _knots64()
    us = np.array([0.15, 0.35, 0.65, 0.85])
    Vinv = np.linalg.inv(np.vander(us, 4, increasing=True))
    g = np.zeros((NJ, 4, NK))
    for j in range(NJ):
        xs = (j + us) / 55.0
        Bs = _basis64(xs, t)                   # (4, 64)
        for ii in range(4):
            coef = Vinv @ Bs[:, j + 3 + ii]    # degree 0..3 in u = s - j
            g[j, :, j + 3 + ii] += coef
    return g


def _make_w2():
    """Constant (65, NW2) float64 matrix W2 such that cpb.T @ W2 gives
    per-feature chain coefficients (cpb = [control_p_shard; bias_shard]).

    Coarse piece k (s in [CW k, CW k + CW)): the cubic of sub-interval CW*k
    extends across the piece; device evaluates via v = 2*clamp01(s/CW - k),
    so coefficients carry (CW/2)^d.  Interior knots m get truncated-power
    corrections e_m * clamp(s-m, 0, K)^3 (K = width to the piece end); the
    device computes v = 2*clamp(s-m, 0, K) and multiplies v^3 by C0, so
    C0 = e_m / 8.

    Columns: k -> a_k; NCP+k -> b_k; 2*NCP+k -> c_k; then corrections in
    knot order; last -> S(0) + bias (bias row = 1).
    """
    g = _gtable()
    w2 = np.zeros((NK + 1, NW2), dtype=np.float64)
    h = CW / 2.0
    for k in range(NCP):
        j = CW * k
        w2[:NK, k] += g[j, 1] * h
        w2[:NK, NCP + k] += g[j, 2] * h ** 2
        w2[:NK, 2 * NCP + k] += g[j, 3] * h ** 3
    col = 3 * NCP
    for m in range(1, NJ):
        if m % CW == 0:
            continue
        e = g[m, 3] - g[m - 1, 3]
        w2[:NK, col] += e / 8.0
        col += 1
    assert col == 3 * NCP + NCOR
    w2[:NK, NW2 - 1] += g[0, 0]     # S(0)
    w2[NK, NW2 - 1] = 1.0           # bias row
    return w2


def _corr_terms():
    """(m, K, col) for the 50 corrections, in W2 column order."""
    out = []
    col = 3 * NCP
    for m in range(1, NJ):
        if m % CW == 0:
            continue
        out.append((m, CW - (m % CW), col))
        col += 1
    return out


# ---------------------------------------------------------------------------
# Custom DVE ops
# ---------------------------------------------------------------------------

def _register_ops():
    """Five 7/8-stage fused ops (v = saturating shifted double-relu):

      AB   : acc + v5*(C0 + C1*v5)      v5 = min(t+|t|, 2),  t = s11 - k
      C    : acc + (v5^2*v5)*C0
      CORR : acc + (v^2*v)*C0           v  = min(t+|t|, C1), t = s - m
      CORRH:       (v^2*v)*C0           chain head, no Src1 (no memset)
      CORRHC:      (v^2*v)*C0 + C1      head + spline const, LIM=2 baked (K=1)
    """
    from concourse import dve_ops
    from concourse.dve_spec import (
        Spec, Src0, Src1, C0, C1, C2, One, minn, sq, lower, Bin, AluOp,
        _has_src1 as has_src1,
    )
    from concourse.dve_uop import DveOpSpec

    names = ["BSPL_AB_ANT", "BSPL_C_ANT", "BSPL_CORR_ANT",
             "BSPL_CORRH_ANT", "BSPL_CORRHC_ANT"]
    if any(op.name == names[0] for op in dve_ops.OPS):
        byname = {op.name: op for op in dve_ops.OPS}
        return [byname[n] for n in names]

    def _vv(in0, imm2, lim):
        tt = in0.astype(np.float32) - np.float32(imm2)
        return np.minimum(tt + np.abs(tt), np.float32(lim)).astype(np.float32)

    t1 = Src0 - C2
    v1 = minn(t1 + Bin(AluOp.ABSOLUTE_VALUE, t1, t1), One + One)
    body_ab = Src1 + v1 * (C0 + C1 * v1)

    def ref_ab(in0, in1, s0, s1, imm2):
        vv = _vv(in0, imm2, 2.0)
        return (in1 + vv * (s0 + s1 * vv)).astype(np.float32)

    t2 = Src0 - C2
    v2 = minn(t2 + Bin(AluOp.ABSOLUTE_VALUE, t2, t2), One + One)
    body_c = Src1 + (sq(v2) * v2) * C0

    def ref_c(in0, in1, s0, s1, imm2):
        vv = _vv(in0, imm2, 2.0)
        return (in1 + (vv * vv * vv) * s0).astype(np.float32)

    t3 = Src0 - C2
    v3 = minn(t3 + Bin(AluOp.ABSOLUTE_VALUE, t3, t3), C1)
    body_corr = Src1 + (sq(v3) * v3) * C0

    def ref_corr(in0, in1, s0, s1, imm2):
        vv = _vv(in0, imm2, s1)
        return (in1 + (vv * vv * vv) * s0).astype(np.float32)

    t4 = Src0 - C2
    v4 = minn(t4 + Bin(AluOp.ABSOLUTE_VALUE, t4, t4), C1)
    body_corrh = (sq(v4) * v4) * C0

    def ref_corrh(in0, in1, s0, s1, imm2):
        vv = _vv(in0, imm2, s1)
        return ((vv * vv * vv) * s0).astype(np.float32)

    t5 = Src0 - C2
    v5 = minn(t5 + Bin(AluOp.ABSOLUTE_VALUE, t5, t5), One + One)
    body_corrhc = (sq(v5) * v5) * C0 + C1

    def ref_corrhc(in0, in1, s0, s1, imm2):
        vv = _vv(in0, imm2, 2.0)
        return ((vv * vv * vv) * s0 + s1).astype(np.float32)

    def _mk(name, spec):
        shas = {}
        for ver in ("v3", "v4"):
            probe = DveOpSpec(name=name, opcode=0,
                              uops=lower(spec, ver=ver), rd1_en=has_src1(spec))
            shas[ver] = probe.sha(ver)
        op = dve_ops.DveOp(name, spec, subdim=False, uops_sha=shas)
        dve_ops.OPS.append(op)
        dve_ops.CUSTOM_DVE_SPECS[name] = spec
        row = dve_ops._CUSTOM_DVE_ROW_BASE + len(dve_ops.OPS) - 1
        assert row < 0x20
        dve_ops._SUB_OPCODE_FOR_NAME[name] = row
        return op

    return [
        _mk("BSPL_AB_ANT", Spec(body=body_ab, reference=ref_ab)),
        _mk("BSPL_C_ANT", Spec(body=body_c, reference=ref_c)),
        _mk("BSPL_CORR_ANT", Spec(body=body_corr, reference=ref_corr)),
        _mk("BSPL_CORRH_ANT", Spec(body=body_corrh, reference=ref_corrh)),
        _mk("BSPL_CORRHC_ANT", Spec(body=body_corrhc, reference=ref_corrhc)),
    ]


# ---------------------------------------------------------------------------
# Bass kernel
# ---------------------------------------------------------------------------

_CACHE = {}


def _schedule():
    """Assign the 60 chain ops to NACC chains.

    Returns (heads, program): heads[i] = (m, K, col) head correction for
    chain i; chain 0's head is a K=1 correction (the CORRHC op bakes LIM=2).
    program = list of (chain, kind, payload) for the remaining ops.  Chains
    2,3 are front-loaded (exhausted first) so their GPSIMD merge overlaps
    the tail of chains 0,1.
    """
    corr = _corr_terms()
    head_idx = [CW - 2] + [(CW - 1) * k for k in range(1, NACC)]  # m=10,12,23
    heads = [corr[i] for i in head_idx]
    rest_corr = [c for i, c in enumerate(corr) if i not in head_idx]
    coarse = []
    for k in range(NCP):
        coarse.append(("ab", k))
        coarse.append(("c", k))
    # interleave: 8 corrections first (s/11 not needed yet), then mix the
    # coarse ops in.
    rest = ([("corr", c) for c in rest_corr[:8]]
            + [x for pair in zip(
                coarse, [("corr", c) for c in rest_corr[8:18]])
               for x in pair]
            + [("corr", c) for c in rest_corr[18:]])
    # chain assignment: 57 rest ops; chains 1,2 finish early so their
    # GPSIMD merge overlaps the tail of chain 0.
    per_chain = [[] for _ in range(NACC)]
    quota = [21, 18, 18]
    ci = 0
    for opspec in rest:
        while len(per_chain[ci]) >= quota[ci]:
            ci = (ci + 1) % NACC
        per_chain[ci].append(opspec)
        ci = (ci + 1) % NACC
    program = []
    pos = [0] * NACC
    while any(pos[i] < len(per_chain[i]) for i in range(NACC)):
        for i in range(NACC):
            if pos[i] < len(per_chain[i]):
                program.append((i, *per_chain[i][pos[i]]))
                pos[i] += 1
    return heads, program


def _build_module(body_reps=1, nj=NJ):
    key = ("nc", body_reps, nj)
    if key in _CACHE:
        return _CACHE[key]
    op_ab, op_c, op_corr, op_corrh, op_corrhc = _register_ops()

    nc = bacc.Bacc("TRN2", target_bir_lowering=False, debug=False,
                   num_devices=NCORES)
    s_in = nc.dram_tensor("s", [FSH, BSH], F32, kind="ExternalInput").ap()
    g_in = nc.dram_tensor("gtab", [FSH, NW2], F32, kind="ExternalInput").ap()
    y_out = nc.dram_tensor("y", [FSH, BSH], F32, kind="ExternalOutput").ap()

    heads, program = _schedule()
    # count per-chain totals to place the early GPSIMD merge
    chain_total = [1] * NACC
    for (ci, _, _) in program:
        chain_total[ci] += 1

    def emit_body(nc, pool, u):
        """One full evaluation: DMA in, 60 chain ops, GPSIMD merges, DMA out.
        Tiles are per-copy (suffix u) so consecutive copies overlap."""
        s_t = pool.tile([FSH, BSH], F32, name=f"s{u}", tag=f"s{u}")
        nc.sync.dma_start(s_t[:], s_in[:])
        gtab = pool.tile([FSH, NW2], F32, name=f"g{u}", tag=f"g{u}")
        nc.sync.dma_start(gtab[:], g_in[:])
        s11 = pool.tile([FSH, BSH], F32, name=f"s11_{u}", tag=f"s11_{u}")
        nc.scalar.mul(s11[:], s_t[:], 1.0 / CW)

        accs = [pool.tile([FSH, BSH], F32, name=f"acc{i}_{u}",
                          tag=f"acc{i}_{u}") for i in range(NACC)]

        def col_ap(col):
            return gtab[:, col:col + 1]

        m, K, col = heads[0]
        assert K == 1
        nc.vector._custom_dve(op_corrhc, out=accs[0][:], in0=s_t[:],
                              s0=col_ap(col), s1=col_ap(NW2 - 1),
                              imm2=float(m))
        for i in range(1, NACC):
            m, K, col = heads[i]
            nc.vector._custom_dve(op_corrh, out=accs[i][:], in0=s_t[:],
                                  s0=col_ap(col), s1=float(2 * K),
                                  imm2=float(m))
        merged12 = False
        done = [1] * NACC
        for (ci, kind, payload) in program:
            t_acc = accs[ci]
            if kind == "corr":
                m, K, col = payload
                nc.vector._custom_dve(op_corr, out=t_acc[:],
                                      in0=s_t[:], in1=t_acc[:],
                                      s0=col_ap(col), s1=float(2 * K),
                                      imm2=float(m))
            elif kind == "ab":
                k = payload
                nc.vector._custom_dve(op_ab, out=t_acc[:], in0=s11[:],
                                      in1=t_acc[:], s0=col_ap(k),
                                      s1=col_ap(NCP + k), imm2=float(k))
            else:  # "c"
                k = payload
                nc.vector._custom_dve(op_c, out=t_acc[:], in0=s11[:],
                                      in1=t_acc[:], s0=col_ap(2 * NCP + k),
                                      imm2=float(k))
            done[ci] += 1
            if (not merged12 and done[1] == chain_total[1]
                    and done[2] == chain_total[2]):
                # chains 1,2 complete: merge on GPSIMD while the DVE
                # finishes chain 0.
                nc.gpsimd.tensor_add(accs[1][:], accs[1][:], accs[2][:])
                merged12 = True
        assert merged12
        nc.gpsimd.tensor_add(accs[0][:], accs[0][:], accs[1][:])
        nc.sync.dma_start(y_out[:], accs[0][:])

    import contextlib
    with tile.TileContext(nc) as tc:
        with contextlib.ExitStack() as _st:
            pool = _st.enter_context(tc.tile_pool(name="p", bufs=1))
            if body_reps == 1:
                emit_body(nc, pool, 0)
            else:
                q, r = divmod(body_reps, UNROLL)
                if q > 0:
                    with tc.For_i(0, q, 1):
                        for u in range(UNROLL):
                            emit_body(nc, pool, u)
                for u in range(r):
                    emit_body(nc, pool, UNROLL + u)

    nc.compile()
    _CACHE[key] = nc
    return nc


# ---------------------------------------------------------------------------
# Public entry point
# ---------------------------------------------------------------------------

def _make_in_maps(x, control_p, bias):
    x = np.ascontiguousarray(x, dtype=np.float32)
    control_p = np.ascontiguousarray(control_p, dtype=np.float32)
    bias = np.ascontiguousarray(bias, dtype=np.float32)
    assert x.shape == (BATCH, NF) and control_p.shape == (NK, NF)
    w2 = _make_w2()            # (65, NW2) float64
    in_maps, slots = [], []
    gtab_cache = {}
    for c in range(NCORES):
        fh, bq = c // 4, c % 4
        fsl = slice(fh * FSH, (fh + 1) * FSH)
        bsl = slice(bq * BSH, (bq + 1) * BSH)
        if fh not in gtab_cache:
            cpb = np.concatenate(
                [control_p[:, fsl], bias[None, fsl]], axis=0)  # (65, 128)
            gtab_cache[fh] = np.ascontiguousarray(
                (cpb.T.astype(np.float64) @ w2).astype(np.float32))
        s_t = np.ascontiguousarray(
            x[bsl, fsl].T * np.float32(55.0), dtype=np.float32)
        in_maps.append({"s": s_t, "gtab": gtab_cache[fh]})
        slots.append((bsl, fsl))
    return in_maps, slots


def kernel(x, control_p, bias):
    nc = _build_module()
    in_maps, slots = _make_in_maps(x, control_p, bias)
    res = run_bass_kernel_spmd(nc, in_maps, list(range(NCORES)))

    out = np.empty((BATCH, NF), dtype=np.float32)
    for c, (bsl, fsl) in enumerate(slots):
        out[bsl, fsl] = res.results[c]["y"].T
    return out


# revision 13
# speedup vs baseline: 1.1411x; 1.1411x over previous
"""Trainium2 Bass kernel for the B-spline (KAN-style) layer.

Computes out[b,f] = sum_k basis_k(x[b,f]) * control_p[k,f] + bias[f] where
basis is the cubic B-spline basis from the reference (64 functions, knots
uniform on [0,1] with spacing 1/55 plus boundary extension knots).

Algorithm: two-level "telescoped clamp" in s = 55*x coordinates (integer
knots).  A C^2 piecewise cubic telescopes into clamped cubics with no
data-dependent lookup:

    S(s) = S(0) + sum_k [coarse cubic_k(clamp(s - 11k, 0, 11))]
                + sum_m e_m * clamp(s - m, 0, K_m)^3

with one width-11 coarse piece per 11 knots (2 fused DVE ops for a,b + c)
and a single-coefficient truncated-power correction at each interior knot m
(clamped at the coarse piece end, K_m in {1..10}).  60 fused 8-stage custom
DVE instructions total.

Unlike the width-5 predecessor, corrections run directly on the raw s tile:
the clamp limit 2*K_m is passed through the second scalar slot
(v = min(t + |t|, C1) = 2*clamp(s-m, 0, K)), so no per-width prescaled
copies of s are needed.  Only the 5 coarse-piece ops use a single s/11 tile
(one ACT mul).  Chain heads are Src1-free op variants (no memsets); the
spline constant + bias rides the C1 slot of one head.

Sharding: data-parallel over batch (4 slices) x features (2 halves) = 8
cores; no collectives.  The host pre-transposes each shard to
(feature, batch) layout and pre-computes the per-feature coefficient table
gtab = [control_p; bias].T @ W2 in float64 (W2 is a fixed host constant),
so the device does no transposes and no table matmul: DMA s + gtab in,
60 chain ops + 3 merge adds (1 on GPSIMD), DMA out.
"""

import sys

if "/opt/trn_rl_repo" not in sys.path:
    sys.path.insert(0, "/opt/trn_rl_repo")

import numpy as np

import concourse.bass as bass
import concourse.bacc as bacc
import concourse.tile as tile
from concourse import mybir
from concourse.bass_utils import run_bass_kernel_spmd

BATCH, NF, NK, DG = 4096, 256, 64, 3
NJ = 55          # spline intervals covering x in [0,1)
NCORES = 8
BSH, FSH = 1024, 128   # per-core shard: batch x features
F32 = mybir.dt.float32

import os as _os

CW = int(_os.environ.get("BSPL_CW", "11"))   # coarse piece width
NCP = -(-NJ // CW)       # coarse pieces (last may extend past s=55; its
                         # corrections then simply never saturate)
NCOR = NJ - NCP          # interior-knot corrections
NW2 = 3 * NCP + NCOR + 1  # a,b,c per coarse piece + corrections + const
NACC = 3                 # independent accumulator chains
UNROLL = 4               # bodies per For_i iteration (amortizes the
                         # all-engine back-edge barrier and lets copy u's
                         # DMA/ACT/GPSIMD head+tail overlap copy u-1's DVE)

import os as _os
_MERGE_ENG = _os.environ.get("BSPL_MERGE", "dve")

# ---------------------------------------------------------------------------
# Host-side spline tables (float64, exact)
# ---------------------------------------------------------------------------

def _knots64():
    dg, nk = DG, NK
    base = np.concatenate([
        np.linspace(-0.002, -0.001, dg),
        np.linspace(0.0, 1.0, nk - 2 * dg - 2),
        np.linspace(1.001, 1.002, dg),
    ])
    dist_lo = base[1] - base[0]
    dist_hi = base[-1] - base[-2]
    left = base[0] - dist_lo * np.arange(dg, 0, -1)
    right = base[-1] + dist_hi * np.arange(1, dg + 1)
    t32 = np.concatenate([left, base, right]).astype(np.float32)
    return t32.astype(np.float64)


def _basis64(x, t):
    xe = x[..., None]
    B = ((t[:-1] <= xe) & (xe < t[1:])).astype(np.float64)
    for k in range(1, DG + 1):
        d1 = t[k:-1] - t[:-k - 1]
        d2 = t[k + 1:] - t[1:-k]
        w1 = np.where(d1 != 0, (xe - t[:-k - 1]) / np.where(d1 != 0, d1, 1.0), 0.0)
        w2 = np.where(d2 != 0, (t[k + 1:] - xe) / np.where(d2 != 0, d2, 1.0), 0.0)
        B = w1 * B[..., :-1] + w2 * B[..., 1:]
    return B  # (..., 64)


def _gtable():
    """Per-interval cubic coefficients g[j, d] as linear maps over the 64
    control points: returns (55, 4, 64) float64."""
    t = _knots64()
    us = np.array([0.15, 0.35, 0.65, 0.85])
    Vinv = np.linalg.inv(np.vander(us, 4, increasing=True))
    g = np.zeros((NJ, 4, NK))
    for j in range(NJ):
        xs = (j + us) / 55.0
        Bs = _basis64(xs, t)                   # (4, 64)
        for ii in range(4):
            coef = Vinv @ Bs[:, j + 3 + ii]    # degree 0..3 in u = s - j
            g[j, :, j + 3 + ii] += coef
    return g


def _make_w2():
    """Constant (65, NW2) float64 matrix W2 such that cpb.T @ W2 gives
    per-feature chain coefficients (cpb = [control_p_shard; bias_shard]).

    Coarse piece k (s in [CW k, CW k + CW)): the cubic of sub-interval CW*k
    extends across the piece; device evaluates via v = 2*clamp01(s/CW - k),
    so coefficients carry (CW/2)^d.  Interior knots m get truncated-power
    corrections e_m * clamp(s-m, 0, K)^3 (K = width to the piece end); the
    device computes v = 2*clamp(s-m, 0, K) and multiplies v^3 by C0, so
    C0 = e_m / 8.

    Columns: k -> a_k; NCP+k -> b_k; 2*NCP+k -> c_k; then corrections in
    knot order; last -> S(0) + bias (bias row = 1).
    """
    g = _gtable()
    w2 = np.zeros((NK + 1, NW2), dtype=np.float64)
    h = CW / 2.0
    for k in range(NCP):
        j = CW * k
        w2[:NK, k] += g[j, 1] * h
        w2[:NK, NCP + k] += g[j, 2] * h ** 2
        w2[:NK, 2 * NCP + k] += g[j, 3] * h ** 3
    col = 3 * NCP
    for m in range(1, NJ):
        if m % CW == 0:
            continue
        e = g[m, 3] - g[m - 1, 3]
        w2[:NK, col] += e / 8.0
        col += 1
    assert col == 3 * NCP + NCOR
    w2[:NK, NW2 - 1] += g[0, 0]     # S(0)
    w2[NK, NW2 - 1] = 1.0           # bias row
    return w2


def _corr_terms():
    """(m, K, col) for the 50 corrections, in W2 column order."""
    out = []
    col = 3 * NCP
    for m in range(1, NJ):
        if m % CW == 0:
            continue
        out.append((m, CW - (m % CW), col))
        col += 1
    return out


# ---------------------------------------------------------------------------
# Custom DVE ops
# ---------------------------------------------------------------------------

def _register_ops():
    """Five 7/8-stage fused ops (v = saturating shifted double-relu):

      AB   : acc + v5*(C0 + C1*v5)      v5 = min(t+|t|, 2),  t = s11 - k
      C    : acc + (v5^2*v5)*C0
      CORR : acc + (v^2*v)*C0           v  = min(t+|t|, C1), t = s - m
      CORRH:       (v^2*v)*C0           chain head, no Src1 (no memset)
      CORRHC:      (v^2*v)*C0 + C1      head + spline const, LIM=2 baked (K=1)
    """
    from concourse import dve_ops
    from concourse.dve_spec import (
        Spec, Src0, Src1, C0, C1, C2, One, minn, sq, lower, Bin, AluOp,
        _has_src1 as has_src1,
    )
    from concourse.dve_uop import DveOpSpec

    names = ["BSPL_AB_ANT", "BSPL_C_ANT", "BSPL_CORR_ANT",
             "BSPL_CORRH_ANT", "BSPL_CORRHC_ANT"]
    if any(op.name == names[0] for op in dve_ops.OPS):
        byname = {op.name: op for op in dve_ops.OPS}
        return [byname[n] for n in names]

    def _vv(in0, imm2, lim):
        tt = in0.astype(np.float32) - np.float32(imm2)
        return np.minimum(tt + np.abs(tt), np.float32(lim)).astype(np.float32)

    t1 = Src0 - C2
    v1 = minn(t1 + Bin(AluOp.ABSOLUTE_VALUE, t1, t1), One + One)
    body_ab = Src1 + v1 * (C0 + C1 * v1)

    def ref_ab(in0, in1, s0, s1, imm2):
        vv = _vv(in0, imm2, 2.0)
        return (in1 + vv * (s0 + s1 * vv)).astype(np.float32)

    t2 = Src0 - C2
    v2 = minn(t2 + Bin(AluOp.ABSOLUTE_VALUE, t2, t2), One + One)
    body_c = Src1 + (sq(v2) * v2) * C0

    def ref_c(in0, in1, s0, s1, imm2):
        vv = _vv(in0, imm2, 2.0)
        return (in1 + (vv * vv * vv) * s0).astype(np.float32)

    t3 = Src0 - C2
    v3 = minn(t3 + Bin(AluOp.ABSOLUTE_VALUE, t3, t3), C1)
    body_corr = Src1 + (sq(v3) * v3) * C0

    def ref_corr(in0, in1, s0, s1, imm2):
        vv = _vv(in0, imm2, s1)
        return (in1 + (vv * vv * vv) * s0).astype(np.float32)

    t4 = Src0 - C2
    v4 = minn(t4 + Bin(AluOp.ABSOLUTE_VALUE, t4, t4), C1)
    body_corrh = (sq(v4) * v4) * C0

    def ref_corrh(in0, in1, s0, s1, imm2):
        vv = _vv(in0, imm2, s1)
        return ((vv * vv * vv) * s0).astype(np.float32)

    t5 = Src0 - C2
    v5 = minn(t5 + Bin(AluOp.ABSOLUTE_VALUE, t5, t5), One + One)
    body_corrhc = (sq(v5) * v5) * C0 + C1

    def ref_corrhc(in0, in1, s0, s1, imm2):
        vv = _vv(in0, imm2, 2.0)
        return ((vv * vv * vv) * s0 + s1).astype(np.float32)

    def _mk(name, spec):
        shas = {}
        for ver in ("v3", "v4"):
            probe = DveOpSpec(name=name, opcode=0,
                              uops=lower(spec, ver=ver), rd1_en=has_src1(spec))
            shas[ver] = probe.sha(ver)
        op = dve_ops.DveOp(name, spec, subdim=False, uops_sha=shas)
        dve_ops.OPS.append(op)
        dve_ops.CUSTOM_DVE_SPECS[name] = spec
        row = dve_ops._CUSTOM_DVE_ROW_BASE + len(dve_ops.OPS) - 1
        assert row < 0x20
        dve_ops._SUB_OPCODE_FOR_NAME[name] = row
        return op

    return [
        _mk("BSPL_AB_ANT", Spec(body=body_ab, reference=ref_ab)),
        _mk("BSPL_C_ANT", Spec(body=body_c, reference=ref_c)),
        _mk("BSPL_CORR_ANT", Spec(body=body_corr, reference=ref_corr)),
        _mk("BSPL_CORRH_ANT", Spec(body=body_corrh, reference=ref_corrh)),
        _mk("BSPL_CORRHC_ANT", Spec(body=body_corrhc, reference=ref_corrhc)),
    ]


# ---------------------------------------------------------------------------
# Bass kernel
# ---------------------------------------------------------------------------

_CACHE = {}


def _schedule():
    """Assign the 60 chain ops to NACC chains.

    Returns (heads, program): heads[i] = (m, K, col) head correction for
    chain i; chain 0's head is a K=1 correction (the CORRHC op bakes LIM=2).
    program = list of (chain, kind, payload) for the remaining ops.  Chains
    2,3 are front-loaded (exhausted first) so their GPSIMD merge overlaps
    the tail of chains 0,1.
    """
    corr = _corr_terms()
    head_idx = [CW - 2] + [(CW - 1) * k for k in range(1, NACC)]  # m=10,12,23
    heads = [corr[i] for i in head_idx]
    rest_corr = [c for i, c in enumerate(corr) if i not in head_idx]
    coarse = []
    for k in range(NCP):
        coarse.append(("ab", k))
        coarse.append(("c", k))
    # interleave: 8 corrections first (s/11 not needed yet), then mix the
    # coarse ops in.
    rest = ([("corr", c) for c in rest_corr[:8]]
            + [x for pair in zip(
                coarse, [("corr", c) for c in rest_corr[8:18]])
               for x in pair]
            + [("corr", c) for c in rest_corr[18:]])
    # chain assignment: 57 rest ops; chains 1,2 finish early so their
    # GPSIMD merge overlaps the tail of chain 0.
    per_chain = [[] for _ in range(NACC)]
    quota = [21, 18, 18]
    ci = 0
    for opspec in rest:
        while len(per_chain[ci]) >= quota[ci]:
            ci = (ci + 1) % NACC
        per_chain[ci].append(opspec)
        ci = (ci + 1) % NACC
    program = []
    pos = [0] * NACC
    while any(pos[i] < len(per_chain[i]) for i in range(NACC)):
        for i in range(NACC):
            if pos[i] < len(per_chain[i]):
                program.append((i, *per_chain[i][pos[i]]))
                pos[i] += 1
    return heads, program


def _build_module(body_reps=1, nj=NJ):
    key = ("nc", body_reps, nj)
    if key in _CACHE:
        return _CACHE[key]
    op_ab, op_c, op_corr, op_corrh, op_corrhc = _register_ops()

    nc = bacc.Bacc("TRN2", target_bir_lowering=False, debug=False,
                   num_devices=NCORES)
    s_in = nc.dram_tensor("s", [FSH, BSH], F32, kind="ExternalInput").ap()
    g_in = nc.dram_tensor("gtab", [FSH, NW2], F32, kind="ExternalInput").ap()
    y_out = nc.dram_tensor("y", [FSH, BSH], F32, kind="ExternalOutput").ap()

    heads, program = _schedule()
    # count per-chain totals to place the early GPSIMD merge
    chain_total = [1] * NACC
    for (ci, _, _) in program:
        chain_total[ci] += 1

    def emit_body(nc, pool, u):
        """One full evaluation: DMA in, 60 chain ops, GPSIMD merges, DMA out.
        Tiles are per-copy (suffix u) so consecutive copies overlap."""
        s_t = pool.tile([FSH, BSH], F32, name=f"s{u}", tag=f"s{u}")
        nc.sync.dma_start(s_t[:], s_in[:])
        gtab = pool.tile([FSH, NW2], F32, name=f"g{u}", tag=f"g{u}")
        nc.sync.dma_start(gtab[:], g_in[:])
        s11 = pool.tile([FSH, BSH], F32, name=f"s11_{u}", tag=f"s11_{u}")
        nc.scalar.mul(s11[:], s_t[:], 1.0 / CW)

        accs = [pool.tile([FSH, BSH], F32, name=f"acc{i}_{u}",
                          tag=f"acc{i}_{u}") for i in range(NACC)]

        def col_ap(col):
            return gtab[:, col:col + 1]

        m, K, col = heads[0]
        assert K == 1
        nc.vector._custom_dve(op_corrhc, out=accs[0][:], in0=s_t[:],
                              s0=col_ap(col), s1=col_ap(NW2 - 1),
                              imm2=float(m))
        for i in range(1, NACC):
            m, K, col = heads[i]
            nc.vector._custom_dve(op_corrh, out=accs[i][:], in0=s_t[:],
                                  s0=col_ap(col), s1=float(2 * K),
                                  imm2=float(m))
        merged12 = False
        done = [1] * NACC
        for (ci, kind, payload) in program:
            t_acc = accs[ci]
            if kind == "corr":
                m, K, col = payload
                nc.vector._custom_dve(op_corr, out=t_acc[:],
                                      in0=s_t[:], in1=t_acc[:],
                                      s0=col_ap(col), s1=float(2 * K),
                                      imm2=float(m))
            elif kind == "ab":
                k = payload
                nc.vector._custom_dve(op_ab, out=t_acc[:], in0=s11[:],
                                      in1=t_acc[:], s0=col_ap(k),
                                      s1=col_ap(NCP + k), imm2=float(k))
            else:  # "c"
                k = payload
                nc.vector._custom_dve(op_c, out=t_acc[:], in0=s11[:],
                                      in1=t_acc[:], s0=col_ap(2 * NCP + k),
                                      imm2=float(k))
            done[ci] += 1
            if (not merged12 and done[1] == chain_total[1]
                    and done[2] == chain_total[2]):
                # chains 1,2 complete: merge them while chain 0 finishes.
                if _MERGE_ENG == "gpsimd":
                    nc.gpsimd.tensor_add(accs[1][:], accs[1][:], accs[2][:])
                else:
                    nc.vector.tensor_add(accs[1][:], accs[1][:], accs[2][:])
                merged12 = True
        assert merged12
        if _MERGE_ENG == "gpsimd":
            nc.gpsimd.tensor_add(accs[0][:], accs[0][:], accs[1][:])
        else:
            nc.vector.tensor_add(accs[0][:], accs[0][:], accs[1][:])
        nc.sync.dma_start(y_out[:], accs[0][:])

    import contextlib
    with tile.TileContext(nc) as tc:
        with contextlib.ExitStack() as _st:
            pool = _st.enter_context(tc.tile_pool(name="p", bufs=1))
            if body_reps == 1:
                emit_body(nc, pool, 0)
            else:
                q, r = divmod(body_reps, UNROLL)
                if q > 0:
                    with tc.For_i(0, q, 1):
                        for u in range(UNROLL):
                            emit_body(nc, pool, u)
                for u in range(r):
                    emit_body(nc, pool, UNROLL + u)

    nc.compile()
    _CACHE[key] = nc
    return nc


# ---------------------------------------------------------------------------
# Public entry point
# ---------------------------------------------------------------------------

def _make_in_maps(x, control_p, bias):
    x = np.ascontiguousarray(x, dtype=np.float32)
    control_p = np.ascontiguousarray(control_p, dtype=np.float32)
    bias = np.ascontiguousarray(bias, dtype=np.float32)
    assert x.shape == (BATCH, NF) and control_p.shape == (NK, NF)
    w2 = _make_w2()            # (65, NW2) float64
    in_maps, slots = [], []
    gtab_cache = {}
    for c in range(NCORES):
        fh, bq = c // 4, c % 4
        fsl = slice(fh * FSH, (fh + 1) * FSH)
        bsl = slice(bq * BSH, (bq + 1) * BSH)
        if fh not in gtab_cache:
            cpb = np.concatenate(
                [control_p[:, fsl], bias[None, fsl]], axis=0)  # (65, 128)
            gtab_cache[fh] = np.ascontiguousarray(
                (cpb.T.astype(np.float64) @ w2).astype(np.float32))
        s_t = np.ascontiguousarray(
            x[bsl, fsl].T * np.float32(55.0), dtype=np.float32)
        in_maps.append({"s": s_t, "gtab": gtab_cache[fh]})
        slots.append((bsl, fsl))
    return in_maps, slots


def kernel(x, control_p, bias):
    nc = _build_module()
    in_maps, slots = _make_in_maps(x, control_p, bias)
    res = run_bass_kernel_spmd(nc, in_maps, list(range(NCORES)))

    out = np.empty((BATCH, NF), dtype=np.float32)
    for c, (bsl, fsl) in enumerate(slots):
        out[bsl, fsl] = res.results[c]["y"].T
    return out


# revision 16
# speedup vs baseline: 1.1584x; 1.0152x over previous
"""Trainium2 Bass kernel for the B-spline (KAN-style) layer.

Computes out[b,f] = sum_k basis_k(x[b,f]) * control_p[k,f] + bias[f] where
basis is the cubic B-spline basis from the reference (64 functions, knots
uniform on [0,1] with spacing 1/55 plus boundary extension knots).

Algorithm: two-level "telescoped clamp" in s = 55*x coordinates (integer
knots).  A C^2 piecewise cubic telescopes into clamped cubics with no
data-dependent lookup:

    S(s) = S(0) + sum_k [coarse cubic_k(clamp(s - 11k, 0, 11))]
                + sum_m e_m * clamp(s - m, 0, K_m)^3

with one width-11 coarse piece per 11 knots (2 fused DVE ops for a,b + c)
and a single-coefficient truncated-power correction at each interior knot m
(clamped at the coarse piece end, K_m in {1..10}).  60 fused 8-stage custom
DVE instructions total.

Unlike the width-5 predecessor, corrections run directly on the raw s tile:
the clamp limit 2*K_m is passed through the second scalar slot
(v = min(t + |t|, C1) = 2*clamp(s-m, 0, K)), so no per-width prescaled
copies of s are needed.  Only the 5 coarse-piece ops use a single s/11 tile
(one ACT mul).  Chain heads are Src1-free op variants (no memsets); the
spline constant + bias rides the C1 slot of one head.

Sharding: data-parallel over batch (4 slices) x features (2 halves) = 8
cores; no collectives.  The host pre-transposes each shard to
(feature, batch) layout and pre-computes the per-feature coefficient table
gtab = [control_p; bias].T @ W2 in float64 (W2 is a fixed host constant),
so the device does no transposes and no table matmul: DMA s + gtab in,
60 chain ops + 3 merge adds (1 on GPSIMD), DMA out.
"""

import sys

if "/opt/trn_rl_repo" not in sys.path:
    sys.path.insert(0, "/opt/trn_rl_repo")

import numpy as np

import concourse.bass as bass
import concourse.bacc as bacc
import concourse.tile as tile
from concourse import mybir
from concourse.bass_utils import run_bass_kernel_spmd

BATCH, NF, NK, DG = 4096, 256, 64, 3
NJ = 55          # spline intervals covering x in [0,1)
NCORES = 8
BSH, FSH = 1024, 128   # per-core shard: batch x features
F32 = mybir.dt.float32

import os as _os

CW = int(_os.environ.get("BSPL_CW", "11"))   # coarse piece width
NCP = -(-NJ // CW)       # coarse pieces (last may extend past s=55; its
                         # corrections then simply never saturate)
NCOR = NJ - NCP          # interior-knot corrections
NW2 = 3 * NCP + NCOR + 1  # a,b,c per coarse piece + corrections + const
NACC = 3                 # independent accumulator chains
UNROLL = 4               # bodies per For_i iteration (amortizes the
                         # all-engine back-edge barrier and lets copy u's
                         # DMA/ACT/GPSIMD head+tail overlap copy u-1's DVE)

import os as _os
_MERGE_ENG = _os.environ.get("BSPL_MERGE", "dve")

# ---------------------------------------------------------------------------
# Host-side spline tables (float64, exact)
# ---------------------------------------------------------------------------

def _knots64():
    dg, nk = DG, NK
    base = np.concatenate([
        np.linspace(-0.002, -0.001, dg),
        np.linspace(0.0, 1.0, nk - 2 * dg - 2),
        np.linspace(1.001, 1.002, dg),
    ])
    dist_lo = base[1] - base[0]
    dist_hi = base[-1] - base[-2]
    left = base[0] - dist_lo * np.arange(dg, 0, -1)
    right = base[-1] + dist_hi * np.arange(1, dg + 1)
    t32 = np.concatenate([left, base, right]).astype(np.float32)
    return t32.astype(np.float64)


def _basis64(x, t):
    xe = x[..., None]
    B = ((t[:-1] <= xe) & (xe < t[1:])).astype(np.float64)
    for k in range(1, DG + 1):
        d1 = t[k:-1] - t[:-k - 1]
        d2 = t[k + 1:] - t[1:-k]
        w1 = np.where(d1 != 0, (xe - t[:-k - 1]) / np.where(d1 != 0, d1, 1.0), 0.0)
        w2 = np.where(d2 != 0, (t[k + 1:] - xe) / np.where(d2 != 0, d2, 1.0), 0.0)
        B = w1 * B[..., :-1] + w2 * B[..., 1:]
    return B  # (..., 64)


def _gtable():
    """Per-interval cubic coefficients g[j, d] as linear maps over the 64
    control points: returns (55, 4, 64) float64."""
    t = _knots64()
    us = np.array([0.15, 0.35, 0.65, 0.85])
    Vinv = np.linalg.inv(np.vander(us, 4, increasing=True))
    g = np.zeros((NJ, 4, NK))
    for j in range(NJ):
        xs = (j + us) / 55.0
        Bs = _basis64(xs, t)                   # (4, 64)
        for ii in range(4):
            coef = Vinv @ Bs[:, j + 3 + ii]    # degree 0..3 in u = s - j
            g[j, :, j + 3 + ii] += coef
    return g


def _make_w2():
    """Constant (65, NW2) float64 matrix W2 such that cpb.T @ W2 gives
    per-feature chain coefficients (cpb = [control_p_shard; bias_shard]).

    Coarse piece k (s in [CW k, CW k + CW)): the cubic of sub-interval CW*k
    extends across the piece; device evaluates via v = 2*clamp01(s/CW - k),
    so coefficients carry (CW/2)^d.  Interior knots m get truncated-power
    corrections e_m * clamp(s-m, 0, K)^3 (K = width to the piece end); the
    device computes v = 2*clamp(s-m, 0, K) and multiplies v^3 by C0, so
    C0 = e_m / 8.

    Columns: k -> a_k; NCP+k -> b_k; 2*NCP+k -> c_k; then corrections in
    knot order; last -> S(0) + bias (bias row = 1).
    """
    g = _gtable()
    w2 = np.zeros((NK + 1, NW2), dtype=np.float64)
    h = CW / 2.0
    for k in range(NCP):
        j = CW * k
        w2[:NK, k] += g[j, 1] * h
        w2[:NK, NCP + k] += g[j, 2] * h ** 2
        w2[:NK, 2 * NCP + k] += g[j, 3] * h ** 3
    col = 3 * NCP
    for m in range(1, NJ):
        if m % CW == 0:
            continue
        e = g[m, 3] - g[m - 1, 3]
        w2[:NK, col] += e / 8.0
        col += 1
    assert col == 3 * NCP + NCOR
    w2[:NK, NW2 - 1] += g[0, 0]     # S(0)
    w2[NK, NW2 - 1] = 1.0           # bias row
    return w2


def _corr_terms():
    """(m, K, col) for the 50 corrections, in W2 column order."""
    out = []
    col = 3 * NCP
    for m in range(1, NJ):
        if m % CW == 0:
            continue
        out.append((m, CW - (m % CW), col))
        col += 1
    return out


# ---------------------------------------------------------------------------
# Custom DVE ops
# ---------------------------------------------------------------------------

def _register_ops():
    """Five 7/8-stage fused ops (v = saturating shifted double-relu):

      AB   : acc + v5*(C0 + C1*v5)      v5 = min(t+|t|, 2),  t = s11 - k
      C    : acc + (v5^2*v5)*C0
      CORR : acc + (v^2*v)*C0           v  = min(t+|t|, C1), t = s - m
      CORRH:       (v^2*v)*C0           chain head, no Src1 (no memset)
      CORRHC:      (v^2*v)*C0 + C1      head + spline const, LIM=2 baked (K=1)
    """
    from concourse import dve_ops
    from concourse.dve_spec import (
        Spec, Src0, Src1, C0, C1, C2, One, minn, sq, lower, Bin, AluOp,
        _has_src1 as has_src1,
    )
    from concourse.dve_uop import DveOpSpec

    names = ["BSPL_AB_ANT", "BSPL_C_ANT", "BSPL_CORR_ANT",
             "BSPL_CORRH_ANT", "BSPL_CORRHC_ANT"]
    if any(op.name == names[0] for op in dve_ops.OPS):
        byname = {op.name: op for op in dve_ops.OPS}
        return [byname[n] for n in names]

    def _vv(in0, imm2, lim):
        tt = in0.astype(np.float32) - np.float32(imm2)
        return np.minimum(tt + np.abs(tt), np.float32(lim)).astype(np.float32)

    t1 = Src0 - C2
    v1 = minn(t1 + Bin(AluOp.ABSOLUTE_VALUE, t1, t1), One + One)
    body_ab = Src1 + v1 * (C0 + C1 * v1)

    def ref_ab(in0, in1, s0, s1, imm2):
        vv = _vv(in0, imm2, 2.0)
        return (in1 + vv * (s0 + s1 * vv)).astype(np.float32)

    t2 = Src0 - C2
    v2 = minn(t2 + Bin(AluOp.ABSOLUTE_VALUE, t2, t2), One + One)
    body_c = Src1 + (sq(v2) * v2) * C0

    def ref_c(in0, in1, s0, s1, imm2):
        vv = _vv(in0, imm2, 2.0)
        return (in1 + (vv * vv * vv) * s0).astype(np.float32)

    t3 = Src0 - C2
    v3 = minn(t3 + Bin(AluOp.ABSOLUTE_VALUE, t3, t3), C1)
    body_corr = Src1 + (sq(v3) * v3) * C0

    def ref_corr(in0, in1, s0, s1, imm2):
        vv = _vv(in0, imm2, s1)
        return (in1 + (vv * vv * vv) * s0).astype(np.float32)

    t4 = Src0 - C2
    v4 = minn(t4 + Bin(AluOp.ABSOLUTE_VALUE, t4, t4), C1)
    body_corrh = (sq(v4) * v4) * C0

    def ref_corrh(in0, in1, s0, s1, imm2):
        vv = _vv(in0, imm2, s1)
        return ((vv * vv * vv) * s0).astype(np.float32)

    t5 = Src0 - C2
    v5 = minn(t5 + Bin(AluOp.ABSOLUTE_VALUE, t5, t5), One + One)
    body_corrhc = (sq(v5) * v5) * C0 + C1

    def ref_corrhc(in0, in1, s0, s1, imm2):
        vv = _vv(in0, imm2, 2.0)
        return ((vv * vv * vv) * s0 + s1).astype(np.float32)

    def _mk(name, spec):
        shas = {}
        for ver in ("v3", "v4"):
            probe = DveOpSpec(name=name, opcode=0,
                              uops=lower(spec, ver=ver), rd1_en=has_src1(spec))
            shas[ver] = probe.sha(ver)
        op = dve_ops.DveOp(name, spec, subdim=False, uops_sha=shas)
        dve_ops.OPS.append(op)
        dve_ops.CUSTOM_DVE_SPECS[name] = spec
        row = dve_ops._CUSTOM_DVE_ROW_BASE + len(dve_ops.OPS) - 1
        assert row < 0x20
        dve_ops._SUB_OPCODE_FOR_NAME[name] = row
        return op

    return [
        _mk("BSPL_AB_ANT", Spec(body=body_ab, reference=ref_ab)),
        _mk("BSPL_C_ANT", Spec(body=body_c, reference=ref_c)),
        _mk("BSPL_CORR_ANT", Spec(body=body_corr, reference=ref_corr)),
        _mk("BSPL_CORRH_ANT", Spec(body=body_corrh, reference=ref_corrh)),
        _mk("BSPL_CORRHC_ANT", Spec(body=body_corrhc, reference=ref_corrhc)),
    ]


# ---------------------------------------------------------------------------
# Bass kernel
# ---------------------------------------------------------------------------

_CACHE = {}


def _schedule():
    """Assign the 60 chain ops to NACC chains.

    Returns (heads, program): heads[i] = (m, K, col) head correction for
    chain i; chain 0's head is a K=1 correction (the CORRHC op bakes LIM=2).
    program = list of (chain, kind, payload) for the remaining ops.  Chains
    2,3 are front-loaded (exhausted first) so their GPSIMD merge overlaps
    the tail of chains 0,1.
    """
    corr = _corr_terms()
    head_idx = [CW - 2] + [(CW - 1) * k for k in range(1, NACC)]  # m=10,12,23
    heads = [corr[i] for i in head_idx]
    rest_corr = [c for i, c in enumerate(corr) if i not in head_idx]
    coarse = []
    for k in range(NCP):
        coarse.append(("ab", k))
        coarse.append(("c", k))
    # interleave: 8 corrections first (s/11 not needed yet), then mix the
    # coarse ops in.
    rest = ([("corr", c) for c in rest_corr[:8]]
            + [x for pair in zip(
                coarse, [("corr", c) for c in rest_corr[8:18]])
               for x in pair]
            + [("corr", c) for c in rest_corr[18:]])
    # chain assignment: 57 rest ops; chains 1,2 finish early so their
    # GPSIMD merge overlaps the tail of chain 0.
    per_chain = [[] for _ in range(NACC)]
    c12 = (len(rest) - 2) // NACC
    quota = [len(rest) - (NACC - 1) * c12] + [c12] * (NACC - 1)
    ci = 0
    for opspec in rest:
        while len(per_chain[ci]) >= quota[ci]:
            ci = (ci + 1) % NACC
        per_chain[ci].append(opspec)
        ci = (ci + 1) % NACC
    program = []
    pos = [0] * NACC
    while any(pos[i] < len(per_chain[i]) for i in range(NACC)):
        for i in range(NACC):
            if pos[i] < len(per_chain[i]):
                program.append((i, *per_chain[i][pos[i]]))
                pos[i] += 1
    return heads, program


def _build_module(body_reps=1, nj=NJ):
    key = ("nc", body_reps, nj)
    if key in _CACHE:
        return _CACHE[key]
    op_ab, op_c, op_corr, op_corrh, op_corrhc = _register_ops()

    nc = bacc.Bacc("TRN2", target_bir_lowering=False, debug=False,
                   num_devices=NCORES)
    s_in = nc.dram_tensor("s", [FSH, BSH], F32, kind="ExternalInput").ap()
    g_in = nc.dram_tensor("gtab", [FSH, NW2], F32, kind="ExternalInput").ap()
    y_out = nc.dram_tensor("y", [FSH, BSH], F32, kind="ExternalOutput").ap()

    heads, program = _schedule()
    # count per-chain totals to place the early GPSIMD merge
    chain_total = [1] * NACC
    for (ci, _, _) in program:
        chain_total[ci] += 1

    def emit_dma_in(nc, pool, u):
        """DMA-ins for copy u, emitted before ALL compute bodies so the SP
        queue is not head-of-line blocked by copy u-1's DMA-out (which waits
        on its final merge and would serialize the copies)."""
        s_t = pool.tile([FSH, BSH], F32, name=f"s{u}", tag=f"s{u}")
        nc.sync.dma_start(s_t[:], s_in[:])
        gtab = pool.tile([FSH, NW2], F32, name=f"g{u}", tag=f"g{u}")
        nc.sync.dma_start(gtab[:], g_in[:])
        return s_t, gtab

    def emit_body(nc, pool, u, s_t, gtab):
        """One evaluation: 60 chain ops, merges, DMA out."""
        s11 = pool.tile([FSH, BSH], F32, name=f"s11_{u}", tag=f"s11_{u}")
        nc.scalar.mul(s11[:], s_t[:], 1.0 / CW)

        accs = [pool.tile([FSH, BSH], F32, name=f"acc{i}_{u}",
                          tag=f"acc{i}_{u}") for i in range(NACC)]

        def col_ap(col):
            return gtab[:, col:col + 1]

        m, K, col = heads[0]
        assert K == 1
        nc.vector._custom_dve(op_corrhc, out=accs[0][:], in0=s_t[:],
                              s0=col_ap(col), s1=col_ap(NW2 - 1),
                              imm2=float(m))
        for i in range(1, NACC):
            m, K, col = heads[i]
            nc.vector._custom_dve(op_corrh, out=accs[i][:], in0=s_t[:],
                                  s0=col_ap(col), s1=float(2 * K),
                                  imm2=float(m))
        merged12 = False
        done = [1] * NACC
        for (ci, kind, payload) in program:
            t_acc = accs[ci]
            if kind == "corr":
                m, K, col = payload
                nc.vector._custom_dve(op_corr, out=t_acc[:],
                                      in0=s_t[:], in1=t_acc[:],
                                      s0=col_ap(col), s1=float(2 * K),
                                      imm2=float(m))
            elif kind == "ab":
                k = payload
                nc.vector._custom_dve(op_ab, out=t_acc[:], in0=s11[:],
                                      in1=t_acc[:], s0=col_ap(k),
                                      s1=col_ap(NCP + k), imm2=float(k))
            else:  # "c"
                k = payload
                nc.vector._custom_dve(op_c, out=t_acc[:], in0=s11[:],
                                      in1=t_acc[:], s0=col_ap(2 * NCP + k),
                                      imm2=float(k))
            done[ci] += 1
            if (not merged12 and done[1] == chain_total[1]
                    and done[2] == chain_total[2]):
                # chains 1,2 complete: merge them while chain 0 finishes.
                if _MERGE_ENG == "gpsimd":
                    nc.gpsimd.tensor_add(accs[1][:], accs[1][:], accs[2][:])
                else:
                    nc.vector.tensor_add(accs[1][:], accs[1][:], accs[2][:])
                merged12 = True
        assert merged12
        if _MERGE_ENG == "gpsimd":
            nc.gpsimd.tensor_add(accs[0][:], accs[0][:], accs[1][:])
        else:
            nc.vector.tensor_add(accs[0][:], accs[0][:], accs[1][:])
        nc.sync.dma_start(y_out[:], accs[0][:])

    import contextlib
    with tile.TileContext(nc) as tc:
        with contextlib.ExitStack() as _st:
            pool = _st.enter_context(tc.tile_pool(name="p", bufs=1))

            def emit_group(us):
                ins = [emit_dma_in(nc, pool, u) for u in us]
                for u, (s_t, gtab) in zip(us, ins):
                    emit_body(nc, pool, u, s_t, gtab)

            if body_reps == 1:
                emit_group([0])
            else:
                q, r = divmod(body_reps, UNROLL)
                if q > 0:
                    with tc.For_i(0, q, 1):
                        emit_group(list(range(UNROLL)))
                if r:
                    emit_group([UNROLL + u for u in range(r)])

    nc.compile()
    _CACHE[key] = nc
    return nc


# ---------------------------------------------------------------------------
# Public entry point
# ---------------------------------------------------------------------------

def _make_in_maps(x, control_p, bias):
    x = np.ascontiguousarray(x, dtype=np.float32)
    control_p = np.ascontiguousarray(control_p, dtype=np.float32)
    bias = np.ascontiguousarray(bias, dtype=np.float32)
    assert x.shape == (BATCH, NF) and control_p.shape == (NK, NF)
    w2 = _make_w2()            # (65, NW2) float64
    in_maps, slots = [], []
    gtab_cache = {}
    for c in range(NCORES):
        fh, bq = c // 4, c % 4
        fsl = slice(fh * FSH, (fh + 1) * FSH)
        bsl = slice(bq * BSH, (bq + 1) * BSH)
        if fh not in gtab_cache:
            cpb = np.concatenate(
                [control_p[:, fsl], bias[None, fsl]], axis=0)  # (65, 128)
            gtab_cache[fh] = np.ascontiguousarray(
                (cpb.T.astype(np.float64) @ w2).astype(np.float32))
        s_t = np.ascontiguousarray(
            x[bsl, fsl].T * np.float32(55.0), dtype=np.float32)
        in_maps.append({"s": s_t, "gtab": gtab_cache[fh]})
        slots.append((bsl, fsl))
    return in_maps, slots


def kernel(x, control_p, bias):
    nc = _build_module()
    in_maps, slots = _make_in_maps(x, control_p, bias)
    res = run_bass_kernel_spmd(nc, in_maps, list(range(NCORES)))

    out = np.empty((BATCH, NF), dtype=np.float32)
    for c, (bsl, fsl) in enumerate(slots):
        out[bsl, fsl] = res.results[c]["y"].T
    return out
